# revision 12
# baseline (speedup 1.0000x reference)
"""Trainium2 Bass kernel for a binarized (1w1a) ResNet BasicBlock.

  out = BN2(bconv3x3(sign(BN1(bconv3x3(sign(x), sign(w1))), g1, b1), sign(w2)), g2, b2) + x

with training-mode BatchNorm over (N, H, W) and identity shortcut.
Shapes: x [64, 256, 28, 28] f32, w [256, 256, 3, 3] f32, g/b [256] f32.

Strategy (8 NeuronCores, data-parallel over batch, 8 images/core):
  - conv3x3 = 9 shifted matmuls over a zero-padded 30x30 spatial layout.
    Activations are sign() in fp8e4 (+-1 exact); contraction over 256 input
    channels runs as a single fp8 DoubleRow matmul (K=128 partitions x 2).
    PSUM accumulates in fp32 -> conv outputs are exact integers.
  - BatchNorm needs global (sync) stats: per-core per-channel sum/sumsq are
    computed on the fly (DVE copy w/ accum_out + ACT Square w/ accum_out),
    then all-reduced across the 8 cores. One tiny AllReduce per channel
    block (4 total) so each block's collective overlaps the other block's
    conv compute; a dummy AllReduce issued at kernel start absorbs the
    expensive first-collective setup (~70us) under the conv1 window.
  - Weights are sign()ed and laid out host-side (negligible: 0.05% of FLOPs).
"""

import os
import sys

sys.path.insert(0, "/opt/trn_rl_repo")

import numpy as np
import ml_dtypes
from contextlib import ExitStack

import concourse.bass as bass
import concourse.tile as tile
from concourse import bacc, mybir
from concourse import bass_utils
from concourse.tile_rust import add_dep_helper

N_CORES = 8
NTOT, C, H, W = 64, 256, 28, 28
NPC = NTOT // N_CORES          # images per core
P, J = 128, 2                  # partition block, channel blocks
PW = 30                        # padded width/height
IMG = PW * PW                  # 900
G = 32                         # guard band (shifted matmul reads +-31)
PLANE = 1060                   # padded plane (>=964); odd stride avoids SBUF bank aliasing
HW = H * W                     # 784
HALF = 392                     # HW // 2, one 15-row psum chunk's interior
CHUNK = 15 * PW                # 450 padded positions per matmul chunk
CNT = float(NTOT * HW)         # BN reduction count: 50176
EPS = 1e-5

F32 = mybir.dt.float32
F16 = mybir.dt.float16
F8 = mybir.dt.float8e4

_cache = {}


def _conv_block(nc, xs, wts, craw, sums, sumsqs, psum, scratch, cb):
    """Binary conv for one output-channel block: 16 psum chunks + stats."""
    for n in range(NPC):
        for half in range(2):
            r0 = half * 15
            acc = psum.tile([P, CHUNK], F32, tag="acc")
            for k in range(9):
                kh, kw = divmod(k, 3)
                base = G + r0 * PW + (kh - 1) * PW + (kw - 1)
                nc.tensor.matmul(
                    acc,
                    lhsT=wts[:, k, :, cb * P:(cb + 1) * P],
                    rhs=xs[:, 2 * n:2 * n + 2, base:base + CHUNK],
                    start=(k == 0),
                    stop=(k == 8),
                    perf_mode=mybir.MatmulPerfMode.DoubleRow,
                )
            rows = acc.rearrange("p (r c) -> p r c", c=PW)
            r_lo = 1 - half  # skip padded row 0 in the first chunk
            intr = rows[:, r_lo:r_lo + 14, 1:1 + W]
            ci = n * 2 + half
            # copy to f16 staging + per-chunk channel sums (DVE)
            nc.vector.tensor_scalar(
                out=craw[:, cb, n, half * HALF:(half + 1) * HALF],
                in0=intr, scalar1=0.0, scalar2=0.0,
                op0=mybir.AluOpType.add, op1=mybir.AluOpType.add,
                accum_out=sums[:, ci:ci + 1],
            )
            # per-chunk channel sum-of-squares (ACT)
            sq = scratch.tile([P, HALF], F32, tag="sq")
            nc.scalar.activation(
                sq, intr, mybir.ActivationFunctionType.Square,
                accum_out=sumsqs[:, ci:ci + 1],
            )


def _bn_coeffs(nc, small, st, g_t, b_t, eps_t, tag):
    """Global-stat BN coefficients: scale = g*rstd, bias = b - mean*scale."""
    mean = small.tile([P, 1], F32, name=f"mean{tag}", tag=f"mean{tag}")
    nc.vector.tensor_scalar_mul(mean, st[:, 0:1], 1.0 / CNT)
    ex2 = small.tile([P, 1], F32, name=f"ex2{tag}", tag=f"ex2{tag}")
    nc.vector.tensor_scalar_mul(ex2, st[:, 1:2], 1.0 / CNT)
    m2 = small.tile([P, 1], F32, name=f"m2{tag}", tag=f"m2{tag}")
    nc.vector.tensor_mul(m2, mean, mean)
    var = small.tile([P, 1], F32, name=f"var{tag}", tag=f"var{tag}")
    nc.vector.tensor_sub(var, ex2, m2)
    sd = small.tile([P, 1], F32, name=f"sd{tag}", tag=f"sd{tag}")
    nc.scalar.activation(sd, var, mybir.ActivationFunctionType.Sqrt, bias=eps_t)
    rstd = small.tile([P, 1], F32, name=f"rstd{tag}", tag=f"rstd{tag}")
    nc.vector.reciprocal(rstd, sd)
    scale = small.tile([P, 1], F32, name=f"scale{tag}", tag=f"scale{tag}")
    nc.vector.tensor_mul(scale, g_t, rstd)
    ms = small.tile([P, 1], F32, name=f"ms{tag}", tag=f"ms{tag}")
    nc.vector.tensor_mul(ms, mean, scale)
    bias = small.tile([P, 1], F32, name=f"bias{tag}", tag=f"bias{tag}")
    nc.vector.tensor_sub(bias, b_t, ms)
    return scale, bias


def _stats_exchange(nc, small, sums, sumsqs, rsem, lsem, rnd, waits, tag):
    """All-reduce [P,2] stats across the 8 cores with one-hot
    remote_dma_broadcast sends (XOR slot assignment) + local slot sum.

    ~2-4us vs ~25us for an ncfw AllReduce (pickup latency + exec).  The
    remote-sem wait threshold is attached post-scheduling (see _build) --
    Tile's single-core scheduling sim cannot see peer increments.
    """
    st = small.tile([P, 2], F32, name=f"arin{tag}", tag=f"arin{tag}")
    nc.vector.reduce_sum(st[:, 0:1], sums, axis=mybir.AxisListType.X)
    nc.vector.reduce_sum(st[:, 1:2], sumsqs, axis=mybir.AxisListType.X)
    slots = small.tile([P, 8, 2], F32, name=f"slots{tag}", tag=f"slots{tag}")
    for i in range(N_CORES):
        rdests = [None] * N_CORES
        rdests[i] = (0, i)
        nc.gpsimd.remote_dma_broadcast(
            out_ap=slots[:, i, :], in_ap=st,
            remote_sem=rsem, local_sem=lsem, rdests=rdests,
        )
    trig = nc.gpsimd.trigger_dma(count=None)
    wg = nc.gpsimd.wait_ge(rsem, 0)
    add_dep_helper(wg.ins, trig.ins, reason="slots wait after trigger")
    waits.append((wg, 16 * (rnd + 1)))
    stg = small.tile([P, 2], F32, name=f"arg{tag}", tag=f"arg{tag}")
    adds = []
    adds.append(nc.gpsimd.tensor_add(stg, slots[:, 0, :], slots[:, 1, :]))
    for i in range(2, N_CORES):
        adds.append(nc.gpsimd.tensor_add(stg, stg, slots[:, i, :]))
    for a in adds:
        add_dep_helper(a.ins, wg.ins, reason="slot read after remote sem wait")
    return stg


def _memset_borders(nc, xs):
    """Zero the guard bands and the 1-px padding border of every plane."""
    nc.vector.memset(xs[:, :, 0:G], 0.0)                         # low guards
    nc.vector.memset(xs[:, :, G + IMG:], 0.0)                    # high guards
    nc.vector.memset(xs[:, :, G:G + PW], 0.0)                    # top rows
    nc.vector.memset(xs[:, :, G + IMG - PW:G + IMG], 0.0)        # bottom rows
    mid = xs[:, :, G + PW:G + IMG - PW].rearrange(
        "p a (r c) -> p a r c", c=PW)
    nc.vector.memset(mid[:, :, :, 0:1], 0.0)                     # left cols
    nc.vector.memset(mid[:, :, :, PW - 1:PW], 0.0)               # right cols


def _build():
    nc = bacc.Bacc("TRN2", target_bir_lowering=False, debug=False,
                   num_devices=N_CORES)
    rsem = nc.alloc_semaphore("rdma_remote")
    lsem = nc.alloc_semaphore("rdma_local")
    waits = []

    x_d = nc.dram_tensor("x", [NPC, C, H, W], F32, kind="ExternalInput").ap()
    w1_d = nc.dram_tensor("w1p", [P, 9, J, C], F8, kind="ExternalInput").ap()
    w2_d = nc.dram_tensor("w2p", [P, 9, J, C], F8, kind="ExternalInput").ap()
    gb1_d = nc.dram_tensor("gb1", [2, J, P], F32, kind="ExternalInput").ap()
    gb2_d = nc.dram_tensor("gb2", [2, J, P], F32, kind="ExternalInput").ap()
    y_d = nc.dram_tensor("y", [NPC, C, H, W], F32, kind="ExternalOutput").ap()

    with tile.TileContext(nc) as tc, ExitStack() as ctx:
        big = ctx.enter_context(tc.tile_pool(name="big", bufs=1))
        small = ctx.enter_context(tc.tile_pool(name="small", bufs=1))
        psum = ctx.enter_context(tc.tile_pool(name="psum", bufs=8, space="PSUM"))
        scratch = ctx.enter_context(tc.tile_pool(name="scratch", bufs=2))
        outp = ctx.enter_context(tc.tile_pool(name="outp", bufs=4))

        # ---- weights for conv1 first, then x (image-major so conv1 can start
        # after the first image's two channel blocks land)
        w1s = big.tile([P, 9, J, C], F8)
        nc.sync.dma_start(out=w1s, in_=w1_d)

        xstage = big.tile([P, J, NPC, HW], F32)
        xs1 = big.tile([P, NPC * J, PLANE], F8)
        xs2 = big.tile([P, NPC * J, PLANE], F8)
        _memset_borders(nc, xs1)
        _memset_borders(nc, xs2)
        for n in range(NPC):
            for j in range(J):
                nc.sync.dma_start(
                    out=xstage[:, j, n, :],
                    in_=x_d[n, j * P:(j + 1) * P].rearrange("p h w -> p (h w)"),
                )
                interior = xs1[:, 2 * n + j, G:G + IMG].rearrange(
                    "p (r c) -> p r c", c=PW)[:, 1:1 + H, 1:1 + W]
                nc.scalar.sign(
                    interior,
                    xstage[:, j, n, :].rearrange("p (r c) -> p r c", c=W),
                )

        w2s = big.tile([P, 9, J, C], F8)
        nc.sync.dma_start(out=w2s, in_=w2_d)
        gb_t = []
        for gb_d in (gb1_d, gb2_d):
            per = []
            for j in range(J):
                g_t = small.tile([P, 1], F32, name=f"g{len(gb_t)}{j}",
                                 tag=f"g{len(gb_t)}{j}")
                b_t = small.tile([P, 1], F32, name=f"b{len(gb_t)}{j}",
                                 tag=f"b{len(gb_t)}{j}")
                nc.sync.dma_start(out=g_t,
                                  in_=gb_d[0, j].rearrange("(p o) -> p o", o=1))
                nc.sync.dma_start(out=b_t,
                                  in_=gb_d[1, j].rearrange("(p o) -> p o", o=1))
                per.append((g_t, b_t))
            gb_t.append(per)
        eps_t = small.tile([P, 1], F32, tag="eps")
        nc.vector.memset(eps_t, EPS)

        # ---- layer 1 (per channel block: conv, stats, AR, coeffs, sign2)
        c1raw = big.tile([P, J, NPC, HW], F16)
        c2raw = big.tile([P, J, NPC, HW], F16)
        for cb in range(2):
            sums = small.tile([P, 16], F32, name=f"s1{cb}", tag=f"s1{cb}")
            sumsqs = small.tile([P, 16], F32, name=f"q1{cb}", tag=f"q1{cb}")
            _conv_block(nc, xs1, w1s, c1raw, sums, sumsqs, psum, scratch, cb)
            stg = _stats_exchange(nc, small, sums, sumsqs, rsem, lsem, cb,
                                  waits, f"1{cb}")
            scale, bias = _bn_coeffs(nc, small, stg, gb_t[0][cb][0],
                                     gb_t[0][cb][1], eps_t, f"1{cb}")
            # interlayer: xs2 <- sign(conv1 * scale + bias), channel block cb
            for n in range(NPC):
                interior = xs2[:, 2 * n + cb, G:G + IMG].rearrange(
                    "p (r c) -> p r c", c=PW)[:, 1:1 + H, 1:1 + W]
                nc.scalar.activation(
                    interior,
                    c1raw[:, cb, n, :].rearrange("p (r c) -> p r c", c=W),
                    mybir.ActivationFunctionType.Sign,
                    bias=bias, scale=scale,
                )

        # ---- layer 2 (per block: conv, stats, AR, coeffs, BN2+shortcut+store)
        for cb in range(2):
            sums = small.tile([P, 16], F32, name=f"s2{cb}", tag=f"s2{cb}")
            sumsqs = small.tile([P, 16], F32, name=f"q2{cb}", tag=f"q2{cb}")
            _conv_block(nc, xs2, w2s, c2raw, sums, sumsqs, psum, scratch, cb)
            stg = _stats_exchange(nc, small, sums, sumsqs, rsem, lsem, 2 + cb,
                                  waits, f"2{cb}")
            scale, bias = _bn_coeffs(nc, small, stg, gb_t[1][cb][0],
                                     gb_t[1][cb][1], eps_t, f"2{cb}")
            for n in range(NPC):
                for half in range(2):
                    sl = slice(half * HALF, (half + 1) * HALF)
                    yt = outp.tile([P, HALF], F32, tag="yt")
                    yo = outp.tile([P, HALF], F32, tag="yo")
                    if half == 0:
                        nc.scalar.activation(
                            yt, c2raw[:, cb, n, sl],
                            mybir.ActivationFunctionType.Identity,
                            bias=bias, scale=scale,
                        )
                        nc.vector.tensor_add(yo, yt, xstage[:, cb, n, sl])
                    else:
                        nc.vector.tensor_scalar(
                            out=yt, in0=c2raw[:, cb, n, sl],
                            scalar1=scale, scalar2=bias,
                            op0=mybir.AluOpType.mult, op1=mybir.AluOpType.add,
                        )
                        nc.vector.tensor_add(yo, yt, xstage[:, cb, n, sl])
                    nc.sync.dma_start(
                        out=y_d[n, cb * P:(cb + 1) * P].rearrange(
                            "p h w -> p (h w)")[:, sl],
                        in_=yo,
                    )

    for wg, thresh in waits:
        wg.wait_op(rsem, thresh, "sem-ge", check=False)
    nc.compile()
    return nc


def _pack_w(w):
    # [co, ci, kh, kw] -> sign -> [ci%128, kh*3+kw, ci//128, co] fp8e4
    s = np.sign(w.astype(np.float32)).reshape(C, J, P, 9)
    return np.ascontiguousarray(s.transpose(2, 3, 1, 0)).astype(
        ml_dtypes.float8_e4m3)


def _pack_gb(g, b):
    return np.ascontiguousarray(
        np.stack([g, b]).astype(np.float32).reshape(2, J, P))


def kernel(x, w1, g1, b1, w2, g2, b2, _profile=False):
    if "nc" not in _cache:
        _cache["nc"] = _build()
    nc = _cache["nc"]

    x = np.ascontiguousarray(x, np.float32)
    w1p, w2p = _pack_w(w1), _pack_w(w2)
    gb1, gb2 = _pack_gb(g1, b1), _pack_gb(g2, b2)
    in_maps = [
        {"x": x[c * NPC:(c + 1) * NPC], "w1p": w1p, "w2p": w2p,
         "gb1": gb1, "gb2": gb2}
        for c in range(N_CORES)
    ]
    res = bass_utils.run_bass_kernel_spmd(
        nc, in_maps, core_ids=list(range(N_CORES)), trace=_profile)
    y = np.concatenate([res.results[c]["y"] for c in range(N_CORES)], axis=0)
    if _profile:
        kernel.last_exec_time_ns = res.exec_time_ns
        kernel.last_results = res
    return y


# revision 13
# speedup vs baseline: 31.3708x; 31.3708x over previous
"""Trainium2 Bass kernel for a binarized (1w1a) ResNet BasicBlock.

  out = BN2(bconv3x3(sign(BN1(bconv3x3(sign(x), sign(w1))), g1, b1), sign(w2)), g2, b2) + x

with training-mode BatchNorm over (N, H, W) and identity shortcut.
Shapes: x [64, 256, 28, 28] f32, w [256, 256, 3, 3] f32, g/b [256] f32.

Strategy (8 NeuronCores, data-parallel over batch, 8 images/core):
  - conv3x3 = 9 shifted matmuls over a zero-padded 30x30 spatial layout.
    Activations are sign() in fp8e4 (+-1 exact); contraction over 256 input
    channels runs as a single fp8 DoubleRow matmul (K=128 partitions x 2).
    PSUM accumulates in fp32 -> conv outputs are exact integers.
  - BatchNorm needs global (sync) stats: per-core per-channel sum/sumsq are
    computed on the fly (DVE copy w/ accum_out + ACT Square w/ accum_out),
    then all-reduced across the 8 cores. One tiny AllReduce per channel
    block (4 total) so each block's collective overlaps the other block's
    conv compute; a dummy AllReduce issued at kernel start absorbs the
    expensive first-collective setup (~70us) under the conv1 window.
  - Weights are sign()ed and laid out host-side (negligible: 0.05% of FLOPs).
"""

import os
import sys

sys.path.insert(0, "/opt/trn_rl_repo")

import numpy as np
import ml_dtypes
from contextlib import ExitStack

import concourse.bass as bass
import concourse.tile as tile
from concourse import bacc, mybir
from concourse import bass_utils
from concourse.tile_rust import add_dep_helper

N_CORES = 8
NTOT, C, H, W = 64, 256, 28, 28
NPC = NTOT // N_CORES          # images per core
P, J = 128, 2                  # partition block, channel blocks
PW = 30                        # padded width/height
IMG = PW * PW                  # 900
G = 32                         # guard band (shifted matmul reads +-31)
PLANE = 1060                   # padded plane (>=964); odd stride avoids SBUF bank aliasing
HW = H * W                     # 784
HALF = 392                     # HW // 2, one 15-row psum chunk's interior
CHUNK = 15 * PW                # 450 padded positions per matmul chunk
CNT = float(NTOT * HW)         # BN reduction count: 50176
EPS = 1e-5

F32 = mybir.dt.float32
F16 = mybir.dt.float16
F8 = mybir.dt.float8e4

_cache = {}


def _conv_block(nc, xs, wts, craw, sums, sumsqs, psum, scratch, cb):
    """Binary conv for one output-channel block: 16 psum chunks + stats."""
    for n in range(NPC):
        for half in range(2):
            r0 = half * 15
            acc = psum.tile([P, CHUNK], F32, tag="acc")
            for k in range(9):
                kh, kw = divmod(k, 3)
                base = G + r0 * PW + (kh - 1) * PW + (kw - 1)
                nc.tensor.matmul(
                    acc,
                    lhsT=wts[:, k, :, cb * P:(cb + 1) * P],
                    rhs=xs[:, 2 * n:2 * n + 2, base:base + CHUNK],
                    start=(k == 0),
                    stop=(k == 8),
                    perf_mode=mybir.MatmulPerfMode.DoubleRow,
                )
            rows = acc.rearrange("p (r c) -> p r c", c=PW)
            r_lo = 1 - half  # skip padded row 0 in the first chunk
            intr = rows[:, r_lo:r_lo + 14, 1:1 + W]
            ci = n * 2 + half
            # copy to f16 staging + per-chunk channel sums (DVE)
            nc.vector.tensor_scalar(
                out=craw[:, cb, n, half * HALF:(half + 1) * HALF],
                in0=intr, scalar1=0.0, scalar2=0.0,
                op0=mybir.AluOpType.add, op1=mybir.AluOpType.add,
                accum_out=sums[:, ci:ci + 1],
            )
            # per-chunk channel sum-of-squares (ACT)
            sq = scratch.tile([P, HALF], F32, tag="sq")
            nc.scalar.activation(
                sq, intr, mybir.ActivationFunctionType.Square,
                accum_out=sumsqs[:, ci:ci + 1],
            )


def _bn_coeffs(nc, small, st, g_t, b_t, eps_t, tag):
    """Global-stat BN coefficients: scale = g*rstd, bias = b - mean*scale."""
    mean = small.tile([P, 1], F32, name=f"mean{tag}", tag=f"mean{tag}")
    nc.vector.tensor_scalar_mul(mean, st[:, 0:1], 1.0 / CNT)
    ex2 = small.tile([P, 1], F32, name=f"ex2{tag}", tag=f"ex2{tag}")
    nc.vector.tensor_scalar_mul(ex2, st[:, 1:2], 1.0 / CNT)
    m2 = small.tile([P, 1], F32, name=f"m2{tag}", tag=f"m2{tag}")
    nc.vector.tensor_mul(m2, mean, mean)
    var = small.tile([P, 1], F32, name=f"var{tag}", tag=f"var{tag}")
    nc.vector.tensor_sub(var, ex2, m2)
    sd = small.tile([P, 1], F32, name=f"sd{tag}", tag=f"sd{tag}")
    nc.scalar.activation(sd, var, mybir.ActivationFunctionType.Sqrt, bias=eps_t)
    rstd = small.tile([P, 1], F32, name=f"rstd{tag}", tag=f"rstd{tag}")
    nc.vector.reciprocal(rstd, sd)
    scale = small.tile([P, 1], F32, name=f"scale{tag}", tag=f"scale{tag}")
    nc.vector.tensor_mul(scale, g_t, rstd)
    ms = small.tile([P, 1], F32, name=f"ms{tag}", tag=f"ms{tag}")
    nc.vector.tensor_mul(ms, mean, scale)
    bias = small.tile([P, 1], F32, name=f"bias{tag}", tag=f"bias{tag}")
    nc.vector.tensor_sub(bias, b_t, ms)
    return scale, bias


def _stats_exchange(nc, small, sums, sumsqs, rsem, lsem, rnd, waits, tag):
    """All-reduce [P,2] stats across the 8 cores with one-hot
    remote_dma_broadcast sends (XOR slot assignment) + local slot sum.

    ~2-4us vs ~25us for an ncfw AllReduce (pickup latency + exec).  The
    remote-sem wait threshold is attached post-scheduling (see _build) --
    Tile's single-core scheduling sim cannot see peer increments.
    """
    st = small.tile([P, 2], F32, name=f"arin{tag}", tag=f"arin{tag}")
    nc.vector.reduce_sum(st[:, 0:1], sums, axis=mybir.AxisListType.X)
    nc.vector.reduce_sum(st[:, 1:2], sumsqs, axis=mybir.AxisListType.X)
    slots = small.tile([P, 8, 2], F32, name=f"slots{tag}", tag=f"slots{tag}")
    for i in range(N_CORES):
        rdests = [None] * N_CORES
        rdests[i] = (0, i)
        nc.gpsimd.remote_dma_broadcast(
            out_ap=slots[:, i, :], in_ap=st,
            remote_sem=rsem, local_sem=lsem, rdests=rdests,
        )
    trig = nc.gpsimd.trigger_dma(count=None)
    wg = nc.gpsimd.wait_ge(rsem, 0)
    add_dep_helper(wg.ins, trig.ins, reason="slots wait after trigger")
    waits.append((wg, 16 * (rnd + 1)))
    stg = small.tile([P, 2], F32, name=f"arg{tag}", tag=f"arg{tag}")
    adds = []
    adds.append(nc.gpsimd.tensor_add(stg, slots[:, 0, :], slots[:, 1, :]))
    for i in range(2, N_CORES):
        adds.append(nc.gpsimd.tensor_add(stg, stg, slots[:, i, :]))
    for a in adds:
        add_dep_helper(a.ins, wg.ins, reason="slot read after remote sem wait")
    return stg


def _memset_borders(nc, xs):
    """Zero the guard bands and the 1-px padding border of every plane."""
    nc.vector.memset(xs[:, :, 0:G], 0.0)                         # low guards
    nc.vector.memset(xs[:, :, G + IMG:], 0.0)                    # high guards
    nc.vector.memset(xs[:, :, G:G + PW], 0.0)                    # top rows
    nc.vector.memset(xs[:, :, G + IMG - PW:G + IMG], 0.0)        # bottom rows
    mid = xs[:, :, G + PW:G + IMG - PW].rearrange(
        "p a (r c) -> p a r c", c=PW)
    nc.vector.memset(mid[:, :, :, 0:1], 0.0)                     # left cols
    nc.vector.memset(mid[:, :, :, PW - 1:PW], 0.0)               # right cols


def _build():
    nc = bacc.Bacc("TRN2", target_bir_lowering=False, debug=False,
                   num_devices=N_CORES)
    rsem = nc.alloc_semaphore("rdma_remote")
    lsem = nc.alloc_semaphore("rdma_local")
    waits = []

    x_d = nc.dram_tensor("x", [NPC, C, H, W], F32, kind="ExternalInput").ap()
    w1_d = nc.dram_tensor("w1p", [P, 9, J, C], F8, kind="ExternalInput").ap()
    w2_d = nc.dram_tensor("w2p", [P, 9, J, C], F8, kind="ExternalInput").ap()
    gb1_d = nc.dram_tensor("gb1", [2, J, P], F32, kind="ExternalInput").ap()
    gb2_d = nc.dram_tensor("gb2", [2, J, P], F32, kind="ExternalInput").ap()
    y_d = nc.dram_tensor("y", [NPC, C, H, W], F32, kind="ExternalOutput").ap()

    with tile.TileContext(nc) as tc, ExitStack() as ctx:
        big = ctx.enter_context(tc.tile_pool(name="big", bufs=1))
        small = ctx.enter_context(tc.tile_pool(name="small", bufs=1))
        psum = ctx.enter_context(tc.tile_pool(name="psum", bufs=8, space="PSUM"))
        scratch = ctx.enter_context(tc.tile_pool(name="scratch", bufs=2))
        outp = ctx.enter_context(tc.tile_pool(name="outp", bufs=4))
        dram = ctx.enter_context(tc.tile_pool(name="dram", bufs=1, space="DRAM"))

        # A single tiny ncfw AllReduce issued first: its real job is to give
        # the 8 cores a coordinated start (independent PJRT dispatch skews
        # launches by milliseconds otherwise, which would stall the
        # remote-DMA stats exchanges). Runs under the input-DMA window.
        zs = small.tile([P, 1], F32, tag="zs")
        nc.vector.memset(zs, 0.0)
        dummy_in = dram.tile([P, 1], F32)
        dummy_out = dram.tile([P, 1], F32)
        nc.sync.dma_start(out=dummy_in, in_=zs)
        nc.gpsimd.collective_compute(
            "AllReduce", mybir.AluOpType.add,
            replica_groups=[list(range(N_CORES))],
            ins=[dummy_in.opt()], outs=[dummy_out.opt()],
        )

        # ---- weights for conv1 first, then x (image-major so conv1 can start
        # after the first image's two channel blocks land)
        w1s = big.tile([P, 9, J, C], F8)
        nc.sync.dma_start(out=w1s, in_=w1_d)

        xstage = big.tile([P, J, NPC, HW], F32)
        xs1 = big.tile([P, NPC * J, PLANE], F8)
        xs2 = big.tile([P, NPC * J, PLANE], F8)
        _memset_borders(nc, xs1)
        _memset_borders(nc, xs2)
        for n in range(NPC):
            for j in range(J):
                nc.sync.dma_start(
                    out=xstage[:, j, n, :],
                    in_=x_d[n, j * P:(j + 1) * P].rearrange("p h w -> p (h w)"),
                )
                interior = xs1[:, 2 * n + j, G:G + IMG].rearrange(
                    "p (r c) -> p r c", c=PW)[:, 1:1 + H, 1:1 + W]
                nc.scalar.sign(
                    interior,
                    xstage[:, j, n, :].rearrange("p (r c) -> p r c", c=W),
                )

        w2s = big.tile([P, 9, J, C], F8)
        nc.sync.dma_start(out=w2s, in_=w2_d)
        gb_t = []
        for gb_d in (gb1_d, gb2_d):
            per = []
            for j in range(J):
                g_t = small.tile([P, 1], F32, name=f"g{len(gb_t)}{j}",
                                 tag=f"g{len(gb_t)}{j}")
                b_t = small.tile([P, 1], F32, name=f"b{len(gb_t)}{j}",
                                 tag=f"b{len(gb_t)}{j}")
                nc.sync.dma_start(out=g_t,
                                  in_=gb_d[0, j].rearrange("(p o) -> p o", o=1))
                nc.sync.dma_start(out=b_t,
                                  in_=gb_d[1, j].rearrange("(p o) -> p o", o=1))
                per.append((g_t, b_t))
            gb_t.append(per)
        eps_t = small.tile([P, 1], F32, tag="eps")
        nc.vector.memset(eps_t, EPS)

        # ---- layer 1 (per channel block: conv, stats, AR, coeffs, sign2)
        c1raw = big.tile([P, J, NPC, HW], F16)
        c2raw = big.tile([P, J, NPC, HW], F16)
        for cb in range(2):
            sums = small.tile([P, 16], F32, name=f"s1{cb}", tag=f"s1{cb}")
            sumsqs = small.tile([P, 16], F32, name=f"q1{cb}", tag=f"q1{cb}")
            _conv_block(nc, xs1, w1s, c1raw, sums, sumsqs, psum, scratch, cb)
            stg = _stats_exchange(nc, small, sums, sumsqs, rsem, lsem, cb,
                                  waits, f"1{cb}")
            scale, bias = _bn_coeffs(nc, small, stg, gb_t[0][cb][0],
                                     gb_t[0][cb][1], eps_t, f"1{cb}")
            # interlayer: xs2 <- sign(conv1 * scale + bias), channel block cb
            for n in range(NPC):
                interior = xs2[:, 2 * n + cb, G:G + IMG].rearrange(
                    "p (r c) -> p r c", c=PW)[:, 1:1 + H, 1:1 + W]
                nc.scalar.activation(
                    interior,
                    c1raw[:, cb, n, :].rearrange("p (r c) -> p r c", c=W),
                    mybir.ActivationFunctionType.Sign,
                    bias=bias, scale=scale,
                )

        # ---- layer 2 (per block: conv, stats, AR, coeffs, BN2+shortcut+store)
        for cb in range(2):
            sums = small.tile([P, 16], F32, name=f"s2{cb}", tag=f"s2{cb}")
            sumsqs = small.tile([P, 16], F32, name=f"q2{cb}", tag=f"q2{cb}")
            _conv_block(nc, xs2, w2s, c2raw, sums, sumsqs, psum, scratch, cb)
            stg = _stats_exchange(nc, small, sums, sumsqs, rsem, lsem, 2 + cb,
                                  waits, f"2{cb}")
            scale, bias = _bn_coeffs(nc, small, stg, gb_t[1][cb][0],
                                     gb_t[1][cb][1], eps_t, f"2{cb}")
            for n in range(NPC):
                for half in range(2):
                    sl = slice(half * HALF, (half + 1) * HALF)
                    yt = outp.tile([P, HALF], F32, tag="yt")
                    yo = outp.tile([P, HALF], F32, tag="yo")
                    if half == 0:
                        nc.scalar.activation(
                            yt, c2raw[:, cb, n, sl],
                            mybir.ActivationFunctionType.Identity,
                            bias=bias, scale=scale,
                        )
                        nc.vector.tensor_add(yo, yt, xstage[:, cb, n, sl])
                    else:
                        nc.vector.tensor_scalar(
                            out=yt, in0=c2raw[:, cb, n, sl],
                            scalar1=scale, scalar2=bias,
                            op0=mybir.AluOpType.mult, op1=mybir.AluOpType.add,
                        )
                        nc.vector.tensor_add(yo, yt, xstage[:, cb, n, sl])
                    nc.sync.dma_start(
                        out=y_d[n, cb * P:(cb + 1) * P].rearrange(
                            "p h w -> p (h w)")[:, sl],
                        in_=yo,
                    )

    for wg, thresh in waits:
        wg.wait_op(rsem, thresh, "sem-ge", check=False)
    nc.compile()
    return nc


def _pack_w(w):
    # [co, ci, kh, kw] -> sign -> [ci%128, kh*3+kw, ci//128, co] fp8e4
    s = np.sign(w.astype(np.float32)).reshape(C, J, P, 9)
    return np.ascontiguousarray(s.transpose(2, 3, 1, 0)).astype(
        ml_dtypes.float8_e4m3)


def _pack_gb(g, b):
    return np.ascontiguousarray(
        np.stack([g, b]).astype(np.float32).reshape(2, J, P))


def kernel(x, w1, g1, b1, w2, g2, b2, _profile=False):
    if "nc" not in _cache:
        _cache["nc"] = _build()
    nc = _cache["nc"]

    x = np.ascontiguousarray(x, np.float32)
    w1p, w2p = _pack_w(w1), _pack_w(w2)
    gb1, gb2 = _pack_gb(g1, b1), _pack_gb(g2, b2)
    in_maps = [
        {"x": x[c * NPC:(c + 1) * NPC], "w1p": w1p, "w2p": w2p,
         "gb1": gb1, "gb2": gb2}
        for c in range(N_CORES)
    ]
    res = bass_utils.run_bass_kernel_spmd(
        nc, in_maps, core_ids=list(range(N_CORES)), trace=_profile)
    y = np.concatenate([res.results[c]["y"] for c in range(N_CORES)], axis=0)
    if _profile:
        kernel.last_exec_time_ns = res.exec_time_ns
        kernel.last_results = res
    return y


# revision 14
# speedup vs baseline: 64.8290x; 2.0665x over previous
"""Trainium2 Bass kernel for a binarized (1w1a) ResNet BasicBlock.

  out = BN2(bconv3x3(sign(BN1(bconv3x3(sign(x), sign(w1))), g1, b1), sign(w2)), g2, b2) + x

with training-mode BatchNorm over (N, H, W) and identity shortcut.
Shapes: x [64, 256, 28, 28] f32, w [256, 256, 3, 3] f32, g/b [256] f32.

Strategy (8 NeuronCores, data-parallel over batch, 8 images/core):
  - conv3x3 = 9 shifted matmuls over a zero-padded 30x30 spatial layout.
    Activations are sign() in fp8e4 (+-1 exact); contraction over 256 input
    channels runs as a single fp8 DoubleRow matmul (K=128 partitions x 2).
    PSUM accumulates in fp32 -> conv outputs are exact integers.
  - BatchNorm needs global (sync) stats: per-core per-channel sum/sumsq are
    computed on the fly (DVE copy w/ accum_out + ACT Square w/ accum_out),
    then all-reduced across the 8 cores. One tiny AllReduce per channel
    block (4 total) so each block's collective overlaps the other block's
    conv compute; a dummy AllReduce issued at kernel start absorbs the
    expensive first-collective setup (~70us) under the conv1 window.
  - Weights are sign()ed and laid out host-side (negligible: 0.05% of FLOPs).
"""

import os
import sys

sys.path.insert(0, "/opt/trn_rl_repo")

import numpy as np
import ml_dtypes
from contextlib import ExitStack

import concourse.bass as bass
import concourse.tile as tile
from concourse import bacc, mybir
from concourse import bass_utils
from concourse.tile_rust import add_dep_helper

N_CORES = 8
NTOT, C, H, W = 64, 256, 28, 28
NPC = NTOT // N_CORES          # images per core
P, J = 128, 2                  # partition block, channel blocks
PW = 30                        # padded width/height
IMG = PW * PW                  # 900
G = 32                         # guard band (shifted matmul reads +-31)
PLANE = 1060                   # padded plane (>=964); odd stride avoids SBUF bank aliasing
HW = H * W                     # 784
HALF = 392                     # HW // 2, one 15-row psum chunk's interior
CHUNK = 15 * PW                # 450 padded positions per matmul chunk
CNT = float(NTOT * HW)         # BN reduction count: 50176
EPS = 1e-5

F32 = mybir.dt.float32
F16 = mybir.dt.float16
F8 = mybir.dt.float8e4

_cache = {}


def _conv_block(nc, xs, wts, craw, sums, sumsqs, psum, scratch, cb):
    """Binary conv for one output-channel block: 16 psum chunks + stats."""
    for n in range(NPC):
        for half in range(2):
            r0 = half * 15
            acc = psum.tile([P, CHUNK], F32, tag="acc")
            for k in range(9):
                kh, kw = divmod(k, 3)
                base = G + r0 * PW + (kh - 1) * PW + (kw - 1)
                nc.tensor.matmul(
                    acc,
                    lhsT=wts[:, k, :, cb * P:(cb + 1) * P],
                    rhs=xs[:, 2 * n:2 * n + 2, base:base + CHUNK],
                    start=(k == 0),
                    stop=(k == 8),
                    perf_mode=mybir.MatmulPerfMode.DoubleRow,
                )
            rows = acc.rearrange("p (r c) -> p r c", c=PW)
            r_lo = 1 - half  # skip padded row 0 in the first chunk
            intr = rows[:, r_lo:r_lo + 14, 1:1 + W]
            ci = n * 2 + half
            # copy to f16 staging + per-chunk channel sums (DVE)
            nc.vector.tensor_scalar(
                out=craw[:, cb, n, half * HALF:(half + 1) * HALF],
                in0=intr, scalar1=0.0, scalar2=0.0,
                op0=mybir.AluOpType.add, op1=mybir.AluOpType.add,
                accum_out=sums[:, ci:ci + 1],
            )
            # per-chunk channel sum-of-squares (ACT)
            sq = scratch.tile([P, HALF], F32, tag="sq")
            nc.scalar.activation(
                sq, intr, mybir.ActivationFunctionType.Square,
                accum_out=sumsqs[:, ci:ci + 1],
            )


def _bn_coeffs(nc, small, st, g_t, b_t, eps_t, tag):
    """Global-stat BN coefficients: scale = g*rstd, bias = b - mean*scale."""
    mean = small.tile([P, 1], F32, name=f"mean{tag}", tag=f"mean{tag}")
    nc.vector.tensor_scalar_mul(mean, st[:, 0:1], 1.0 / CNT)
    ex2 = small.tile([P, 1], F32, name=f"ex2{tag}", tag=f"ex2{tag}")
    nc.vector.tensor_scalar_mul(ex2, st[:, 1:2], 1.0 / CNT)
    m2 = small.tile([P, 1], F32, name=f"m2{tag}", tag=f"m2{tag}")
    nc.vector.tensor_mul(m2, mean, mean)
    var = small.tile([P, 1], F32, name=f"var{tag}", tag=f"var{tag}")
    nc.vector.tensor_sub(var, ex2, m2)
    sd = small.tile([P, 1], F32, name=f"sd{tag}", tag=f"sd{tag}")
    nc.scalar.activation(sd, var, mybir.ActivationFunctionType.Sqrt, bias=eps_t)
    rstd = small.tile([P, 1], F32, name=f"rstd{tag}", tag=f"rstd{tag}")
    nc.vector.reciprocal(rstd, sd)
    scale = small.tile([P, 1], F32, name=f"scale{tag}", tag=f"scale{tag}")
    nc.vector.tensor_mul(scale, g_t, rstd)
    ms = small.tile([P, 1], F32, name=f"ms{tag}", tag=f"ms{tag}")
    nc.vector.tensor_mul(ms, mean, scale)
    bias = small.tile([P, 1], F32, name=f"bias{tag}", tag=f"bias{tag}")
    nc.vector.tensor_sub(bias, b_t, ms)
    return scale, bias


def _stats_ar(nc, small, dram, sums, sumsqs, tag):
    """Reduce per-chunk stats, ncfw-all-reduce across cores, return [P,2]."""
    st = small.tile([P, 2], F32, name=f"arin{tag}", tag=f"arin{tag}")
    nc.vector.reduce_sum(st[:, 0:1], sums, axis=mybir.AxisListType.X)
    nc.vector.reduce_sum(st[:, 1:2], sumsqs, axis=mybir.AxisListType.X)
    ar_in = dram.tile([P, 2], F32, name=f"ari{tag}")
    ar_out = dram.tile([P, 2], F32, name=f"aro{tag}")
    nc.gpsimd.dma_start(out=ar_in, in_=st)
    nc.gpsimd.collective_compute(
        "AllReduce", mybir.AluOpType.add,
        replica_groups=[list(range(N_CORES))],
        ins=[ar_in.opt()], outs=[ar_out.opt()],
    )
    stg = small.tile([P, 2], F32, name=f"arg{tag}", tag=f"arg{tag}")
    nc.gpsimd.dma_start(out=stg, in_=ar_out)
    return stg


def _memset_borders(nc, xs):
    """Zero the guard bands and the 1-px padding border of every plane."""
    nc.vector.memset(xs[:, :, 0:G], 0.0)                         # low guards
    nc.vector.memset(xs[:, :, G + IMG:], 0.0)                    # high guards
    nc.vector.memset(xs[:, :, G:G + PW], 0.0)                    # top rows
    nc.vector.memset(xs[:, :, G + IMG - PW:G + IMG], 0.0)        # bottom rows
    mid = xs[:, :, G + PW:G + IMG - PW].rearrange(
        "p a (r c) -> p a r c", c=PW)
    nc.vector.memset(mid[:, :, :, 0:1], 0.0)                     # left cols
    nc.vector.memset(mid[:, :, :, PW - 1:PW], 0.0)               # right cols


def _build():
    nc = bacc.Bacc("TRN2", target_bir_lowering=False, debug=False,
                   num_devices=N_CORES)


    x_d = nc.dram_tensor("x", [NPC, C, H, W], F32, kind="ExternalInput").ap()
    w1_d = nc.dram_tensor("w1p", [P, 9, J, C], F8, kind="ExternalInput").ap()
    w2_d = nc.dram_tensor("w2p", [P, 9, J, C], F8, kind="ExternalInput").ap()
    gb1_d = nc.dram_tensor("gb1", [2, J, P], F32, kind="ExternalInput").ap()
    gb2_d = nc.dram_tensor("gb2", [2, J, P], F32, kind="ExternalInput").ap()
    y_d = nc.dram_tensor("y", [NPC, C, H, W], F32, kind="ExternalOutput").ap()

    with tile.TileContext(nc) as tc, ExitStack() as ctx:
        big = ctx.enter_context(tc.tile_pool(name="big", bufs=1))
        small = ctx.enter_context(tc.tile_pool(name="small", bufs=1))
        psum = ctx.enter_context(tc.tile_pool(name="psum", bufs=8, space="PSUM"))
        scratch = ctx.enter_context(tc.tile_pool(name="scratch", bufs=2))
        outp = ctx.enter_context(tc.tile_pool(name="outp", bufs=4))
        dram = ctx.enter_context(tc.tile_pool(name="dram", bufs=1, space="DRAM"))

        # A single tiny ncfw AllReduce issued first: its real job is to give
        # the 8 cores a coordinated start (independent PJRT dispatch skews
        # launches by milliseconds otherwise, which would stall the
        # remote-DMA stats exchanges). Runs under the input-DMA window.
        zs = small.tile([P, 1], F32, tag="zs")
        nc.vector.memset(zs, 0.0)
        dummy_in = dram.tile([P, 1], F32)
        dummy_out = dram.tile([P, 1], F32)
        nc.sync.dma_start(out=dummy_in, in_=zs)
        nc.gpsimd.collective_compute(
            "AllReduce", mybir.AluOpType.add,
            replica_groups=[list(range(N_CORES))],
            ins=[dummy_in.opt()], outs=[dummy_out.opt()],
        )

        # ---- weights for conv1 first, then x (image-major so conv1 can start
        # after the first image's two channel blocks land)
        w1s = big.tile([P, 9, J, C], F8)
        nc.sync.dma_start(out=w1s, in_=w1_d)

        xstage = big.tile([P, J, NPC, HW], F32)
        xs1 = big.tile([P, NPC * J, PLANE], F8)
        xs2 = big.tile([P, NPC * J, PLANE], F8)
        _memset_borders(nc, xs1)
        _memset_borders(nc, xs2)
        for n in range(NPC):
            for j in range(J):
                nc.sync.dma_start(
                    out=xstage[:, j, n, :],
                    in_=x_d[n, j * P:(j + 1) * P].rearrange("p h w -> p (h w)"),
                )
                interior = xs1[:, 2 * n + j, G:G + IMG].rearrange(
                    "p (r c) -> p r c", c=PW)[:, 1:1 + H, 1:1 + W]
                nc.scalar.sign(
                    interior,
                    xstage[:, j, n, :].rearrange("p (r c) -> p r c", c=W),
                )

        w2s = big.tile([P, 9, J, C], F8)
        nc.sync.dma_start(out=w2s, in_=w2_d)
        gb_t = []
        for gb_d in (gb1_d, gb2_d):
            per = []
            for j in range(J):
                g_t = small.tile([P, 1], F32, name=f"g{len(gb_t)}{j}",
                                 tag=f"g{len(gb_t)}{j}")
                b_t = small.tile([P, 1], F32, name=f"b{len(gb_t)}{j}",
                                 tag=f"b{len(gb_t)}{j}")
                nc.sync.dma_start(out=g_t,
                                  in_=gb_d[0, j].rearrange("(p o) -> p o", o=1))
                nc.sync.dma_start(out=b_t,
                                  in_=gb_d[1, j].rearrange("(p o) -> p o", o=1))
                per.append((g_t, b_t))
            gb_t.append(per)
        eps_t = small.tile([P, 1], F32, tag="eps")
        nc.vector.memset(eps_t, EPS)

        # ---- layer 1 (per channel block: conv, stats, AR, coeffs, sign2)
        c1raw = big.tile([P, J, NPC, HW], F16)
        c2raw = big.tile([P, J, NPC, HW], F16)
        for cb in range(2):
            sums = small.tile([P, 16], F32, name=f"s1{cb}", tag=f"s1{cb}")
            sumsqs = small.tile([P, 16], F32, name=f"q1{cb}", tag=f"q1{cb}")
            _conv_block(nc, xs1, w1s, c1raw, sums, sumsqs, psum, scratch, cb)
            stg = _stats_ar(nc, small, dram, sums, sumsqs, f"1{cb}")
            scale, bias = _bn_coeffs(nc, small, stg, gb_t[0][cb][0],
                                     gb_t[0][cb][1], eps_t, f"1{cb}")
            # interlayer: xs2 <- sign(conv1 * scale + bias), channel block cb
            for n in range(NPC):
                interior = xs2[:, 2 * n + cb, G:G + IMG].rearrange(
                    "p (r c) -> p r c", c=PW)[:, 1:1 + H, 1:1 + W]
                nc.scalar.activation(
                    interior,
                    c1raw[:, cb, n, :].rearrange("p (r c) -> p r c", c=W),
                    mybir.ActivationFunctionType.Sign,
                    bias=bias, scale=scale,
                )

        # ---- layer 2 (per block: conv, stats, AR, coeffs, BN2+shortcut+store)
        for cb in range(2):
            sums = small.tile([P, 16], F32, name=f"s2{cb}", tag=f"s2{cb}")
            sumsqs = small.tile([P, 16], F32, name=f"q2{cb}", tag=f"q2{cb}")
            _conv_block(nc, xs2, w2s, c2raw, sums, sumsqs, psum, scratch, cb)
            stg = _stats_ar(nc, small, dram, sums, sumsqs, f"2{cb}")
            scale, bias = _bn_coeffs(nc, small, stg, gb_t[1][cb][0],
                                     gb_t[1][cb][1], eps_t, f"2{cb}")
            for n in range(NPC):
                for half in range(2):
                    sl = slice(half * HALF, (half + 1) * HALF)
                    yt = outp.tile([P, HALF], F32, tag="yt")
                    yo = outp.tile([P, HALF], F32, tag="yo")
                    if half == 0:
                        nc.scalar.activation(
                            yt, c2raw[:, cb, n, sl],
                            mybir.ActivationFunctionType.Identity,
                            bias=bias, scale=scale,
                        )
                        nc.vector.tensor_add(yo, yt, xstage[:, cb, n, sl])
                    else:
                        nc.vector.tensor_scalar(
                            out=yt, in0=c2raw[:, cb, n, sl],
                            scalar1=scale, scalar2=bias,
                            op0=mybir.AluOpType.mult, op1=mybir.AluOpType.add,
                        )
                        nc.vector.tensor_add(yo, yt, xstage[:, cb, n, sl])
                    nc.sync.dma_start(
                        out=y_d[n, cb * P:(cb + 1) * P].rearrange(
                            "p h w -> p (h w)")[:, sl],
                        in_=yo,
                    )

    nc.compile()
    return nc


def _pack_w(w):
    # [co, ci, kh, kw] -> sign -> [ci%128, kh*3+kw, ci//128, co] fp8e4
    s = np.sign(w.astype(np.float32)).reshape(C, J, P, 9)
    return np.ascontiguousarray(s.transpose(2, 3, 1, 0)).astype(
        ml_dtypes.float8_e4m3)


def _pack_gb(g, b):
    return np.ascontiguousarray(
        np.stack([g, b]).astype(np.float32).reshape(2, J, P))


def kernel(x, w1, g1, b1, w2, g2, b2, _profile=False):
    if "nc" not in _cache:
        _cache["nc"] = _build()
    nc = _cache["nc"]

    x = np.ascontiguousarray(x, np.float32)
    w1p, w2p = _pack_w(w1), _pack_w(w2)
    gb1, gb2 = _pack_gb(g1, b1), _pack_gb(g2, b2)
    in_maps = [
        {"x": x[c * NPC:(c + 1) * NPC], "w1p": w1p, "w2p": w2p,
         "gb1": gb1, "gb2": gb2}
        for c in range(N_CORES)
    ]
    res = bass_utils.run_bass_kernel_spmd(
        nc, in_maps, core_ids=list(range(N_CORES)), trace=_profile)
    y = np.concatenate([res.results[c]["y"] for c in range(N_CORES)], axis=0)
    if _profile:
        kernel.last_exec_time_ns = res.exec_time_ns
        kernel.last_results = res
    return y
